# revision 3
# baseline (speedup 1.0000x reference)
"""Distributed Trainium2 kernel for the attention block (8 NeuronCores).

Sharding: core i owns KV head i and Q heads [4i, 4i+4) (tensor parallel over
heads). Attention outputs are exchanged with an AllToAll so each core computes
the o_proj for its 256-row slice of the sequence; host concatenates slices.

All matmuls run in bf16 with fp32 PSUM accumulation; softmax statistics and
norms in fp32. RoPE + QK RMS-norm are folded into precomputed tables and
per-row scales.
"""
import numpy as np
import ml_dtypes

import concourse.bass as bass
import concourse.bacc as bacc
import concourse.tile as tile
from concourse import mybir
from concourse.bass_utils import run_bass_kernel_spmd

BF16 = mybir.dt.bfloat16
F32 = mybir.dt.float32

N_CORES = 8
S = 2048
D_MODEL = 2560
HEAD_DIM = 128
NUM_HEADS = 32
NUM_KV_HEADS = 8
Q_PER_CORE = NUM_HEADS // N_CORES          # 4
QKV_R = Q_PER_CORE * HEAD_DIM + 2 * HEAD_DIM  # 768 rows per core (4q + k + v)
QK_R = Q_PER_CORE * HEAD_DIM + HEAD_DIM       # 640 (q + k, normed+roped)
ROPE_THETA = 5e6
EPS = 1e-6
N_ST = S // 128          # 16 seq tiles
N_DC = D_MODEL // 128    # 20 contraction chunks
S_SLICE = S // N_CORES   # 256 output rows per core
N_HD = (NUM_HEADS * HEAD_DIM) // 128  # 32 o_proj contraction chunks
AV_W = 132               # 128 v cols + 1 ones + 3 pad

_CACHE = {}


def _build():
    nc = bacc.Bacc("TRN2", target_bir_lowering=False, debug=False,
                   num_devices=N_CORES)

    xT = nc.dram_tensor("xT", [D_MODEL, S], BF16, kind="ExternalInput")
    wqkvT = nc.dram_tensor("wqkvT", [D_MODEL, QKV_R], BF16, kind="ExternalInput")
    woT = nc.dram_tensor("woT", [NUM_HEADS * HEAD_DIM, D_MODEL], BF16,
                         kind="ExternalInput")
    cosT = nc.dram_tensor("cosT", [S, QK_R], BF16, kind="ExternalInput")
    sinT = nc.dram_tensor("sinT", [S, QK_R], BF16, kind="ExternalInput")
    tri = nc.dram_tensor("tri", [128, 128], BF16, kind="ExternalInput")

    out_slice = nc.dram_tensor("out_slice", [S_SLICE, D_MODEL], F32,
                               kind="ExternalOutput")
    k_cache = nc.dram_tensor("k_cache", [S, HEAD_DIM], F32, kind="ExternalOutput")
    v_cache = nc.dram_tensor("v_cache", [S, HEAD_DIM], F32, kind="ExternalOutput")

    with tile.TileContext(nc) as tc:
        with (
            tc.tile_pool(name="persist", bufs=1) as pp,
            tc.tile_pool(name="qkt", bufs=5) as qkt_pool,
            tc.tile_pool(name="vaug", bufs=N_ST) as vaug_pool,
            tc.tile_pool(name="dram", bufs=1, space="DRAM") as dram,
        ):
            # persistent tiles
            qkT = [qkt_pool.tile([128, S], BF16, tag="qkt", name=f"qkT{b}")
                   for b in range(5)]
            v_aug = [vaug_pool.tile([128, AV_W], BF16, tag="vaug", name=f"vaug{t}")
                     for t in range(N_ST)]
            tri_sb = pp.tile([128, 128], BF16)
            nc.sync.dma_start(out=tri_sb[:], in_=tri[:])
            eps_sb = pp.tile([128, 1], F32)
            nc.vector.memset(eps_sb, float(HEAD_DIM) * EPS)
            for t in range(N_ST):
                nc.gpsimd.memset(v_aug[t][:, 128:129], 1.0)
                nc.gpsimd.memset(v_aug[t][:, 129:AV_W], 0.0)

            a2a_in = dram.tile([N_CORES, S_SLICE, Q_PER_CORE * HEAD_DIM], BF16)
            a2a_out = dram.tile([N_CORES, S_SLICE, Q_PER_CORE * HEAD_DIM], BF16)

            # ---------------- Phase A: QKV projection + norm + rope ----------
            with (
                tc.tile_pool(name="xt", bufs=N_DC) as xt_pool,
                tc.tile_pool(name="wk", bufs=N_DC) as wk_pool,
                tc.tile_pool(name="tabs", bufs=3) as tab_pool,
                tc.tile_pool(name="qkwork", bufs=3) as work_pool,
                tc.tile_pool(name="stats", bufs=4) as stat_pool,
                tc.tile_pool(name="apsum", bufs=2, space="PSUM") as apsum,
            ):
                xT_sb = [xt_pool.tile([128, S], BF16, tag="xt", name=f"xTs{d}")
                         for d in range(N_DC)]
                wq_sb = [wk_pool.tile([128, QKV_R], BF16, tag="wk", name=f"wqs{d}")
                         for d in range(N_DC)]
                for d in range(N_DC):
                    nc.sync.dma_start(out=xT_sb[d][:], in_=xT[128 * d:128 * (d + 1), :])
                    nc.sync.dma_start(out=wq_sb[d][:], in_=wqkvT[128 * d:128 * (d + 1), :])

                for t in range(N_ST):
                    ps = apsum.tile([128, QKV_R], F32, tag="qkvps")
                    for d in range(N_DC):
                        lhsT = xT_sb[d][:, 128 * t:128 * (t + 1)]
                        nc.tensor.matmul(ps[:, 0:512], lhsT, wq_sb[d][:, 0:512],
                                         start=(d == 0), stop=(d == N_DC - 1))
                        nc.tensor.matmul(ps[:, 512:QKV_R], lhsT, wq_sb[d][:, 512:QKV_R],
                                         start=(d == 0), stop=(d == N_DC - 1))

                    # v: fp32 cache out + bf16 for AV
                    v32 = work_pool.tile([128, HEAD_DIM], F32, tag="v32")
                    nc.vector.tensor_copy(v32[:], ps[:, 640:768])
                    nc.sync.dma_start(out=v_cache[128 * t:128 * (t + 1), :], in_=v32[:])
                    nc.vector.tensor_copy(v_aug[t][:, 0:128], v32[:])

                    # rms-norm stats for 4 q heads + k
                    sums = stat_pool.tile([128, 5], F32, tag="sums")
                    scr = work_pool.tile([128, 128], BF16, tag="sqscr")
                    for b in range(5):
                        nc.scalar.activation(
                            out=scr[:], in_=ps[:, 128 * b:128 * (b + 1)],
                            func=mybir.ActivationFunctionType.Square,
                            accum_out=sums[:, b:b + 1])
                    inv = stat_pool.tile([128, 5], F32, tag="inv")
                    nc.scalar.activation(out=inv[:], in_=sums[:],
                                         func=mybir.ActivationFunctionType.Sqrt,
                                         bias=eps_sb[:])
                    nc.vector.reciprocal(inv[:], inv[:])
                    # k scale needs * sqrt(HEAD_DIM) (q keeps 1/sqrt(d) fold)
                    nc.vector.tensor_scalar_mul(inv[:, 4:5], inv[:, 4:5],
                                                float(np.sqrt(HEAD_DIM)))

                    # evacuate q,k with norm scale applied -> bf16
                    qk_n = work_pool.tile([128, QK_R], BF16, tag="qkn")
                    for b in range(5):
                        nc.scalar.activation(
                            out=qk_n[:, 128 * b:128 * (b + 1)],
                            in_=ps[:, 128 * b:128 * (b + 1)],
                            func=mybir.ActivationFunctionType.Copy,
                            scale=inv[:, b:b + 1])

                    # rope
                    cos_sb = tab_pool.tile([128, QK_R], BF16, tag="cos")
                    sin_sb = tab_pool.tile([128, QK_R], BF16, tag="sin")
                    nc.sync.dma_start(out=cos_sb[:], in_=cosT[128 * t:128 * (t + 1), :])
                    nc.sync.dma_start(out=sin_sb[:], in_=sinT[128 * t:128 * (t + 1), :])
                    qk_r = work_pool.tile([128, QK_R], BF16, tag="qkr")
                    tmp = work_pool.tile([128, QK_R], BF16, tag="ropetmp")
                    qn3 = qk_n.rearrange("p (b h) -> p b h", b=5)
                    qr3 = qk_r.rearrange("p (b h) -> p b h", b=5)
                    sn3 = sin_sb.rearrange("p (b h) -> p b h", b=5)
                    nc.vector.tensor_mul(qr3[:, :, 0:64], qn3[:, :, 64:128],
                                         sn3[:, :, 0:64])
                    nc.vector.tensor_mul(qr3[:, :, 64:128], qn3[:, :, 0:64],
                                         sn3[:, :, 64:128])
                    nc.vector.tensor_mul(tmp[:], qk_n[:], cos_sb[:])
                    nc.vector.tensor_add(qk_r[:], qk_r[:], tmp[:])

                    # k cache (fp32)
                    k32 = work_pool.tile([128, HEAD_DIM], F32, tag="k32")
                    nc.vector.tensor_copy(k32[:], qk_r[:, 512:640])
                    nc.sync.dma_start(out=k_cache[128 * t:128 * (t + 1), :], in_=k32[:])

                    # transpose q heads + k into qkT tiles
                    for b in range(5):
                        nc.sync.dma_start_transpose(
                            out=qkT[b][:, 128 * t:128 * (t + 1)],
                            in_=qk_r[:, 128 * b:128 * (b + 1)])

            # ---------------- Phase B: attention per head -------------------
            with (
                tc.tile_pool(name="pt", bufs=1) as pt_pool,
                tc.tile_pool(name="asb", bufs=3) as a_pool,
                tc.tile_pool(name="den", bufs=4) as den_pool,
                tc.tile_pool(name="spsum", bufs=3, space="PSUM") as spsum,
                tc.tile_pool(name="avpsum", bufs=3, space="PSUM") as avpsum,
            ):
                kT = qkT[4]
                for h in range(Q_PER_CORE):
                    qT = qkT[h]
                    strips = []
                    for c in range(N_ST):
                        n_sq = S - 128 * c
                        pt = pt_pool.tile([128, n_sq], BF16, tag=f"pt{c}")
                        strips.append(pt)
                        for j in range(0, n_sq, 512):
                            w = min(512, n_sq - j)
                            sps = spsum.tile([128, 512], F32, tag="sps")
                            nc.tensor.matmul(sps[:, 0:w],
                                             kT[:, 128 * c:128 * (c + 1)],
                                             qT[:, 128 * c + j:128 * c + j + w],
                                             start=True, stop=True)
                            nc.scalar.activation(out=pt[:, j:j + w], in_=sps[:, 0:w],
                                                 func=mybir.ActivationFunctionType.Exp)
                        # causal mask on diagonal block (keep sk <= sq)
                        nc.gpsimd.tensor_mul(pt[:, 0:128], pt[:, 0:128], tri_sb[:])

                    for t in range(N_ST):
                        av = avpsum.tile([128, AV_W], F32, tag="av")
                        for c in range(t + 1):
                            nc.tensor.matmul(av[:],
                                             strips[c][:, 128 * (t - c):128 * (t - c + 1)],
                                             v_aug[c][:],
                                             start=(c == 0), stop=(c == t))
                        den = den_pool.tile([128, 1], F32, tag="den")
                        nc.vector.reciprocal(den[:], av[:, 128:129])
                        a_sb = a_pool.tile([128, HEAD_DIM], BF16, tag="asb")
                        nc.scalar.activation(out=a_sb[:], in_=av[:, 0:128],
                                             func=mybir.ActivationFunctionType.Copy,
                                             scale=den[:])
                        nc.sync.dma_start(
                            out=a2a_in[t // 2,
                                       128 * (t % 2):128 * (t % 2 + 1),
                                       128 * h:128 * (h + 1)],
                            in_=a_sb[:])

                nc.gpsimd.collective_compute(
                    "AllToAll", mybir.AluOpType.bypass,
                    replica_groups=[list(range(N_CORES))],
                    ins=[a2a_in.opt()], outs=[a2a_out.opt()])

            # ---------------- Phase C: o_proj for our seq slice -------------
            with (
                tc.tile_pool(name="at", bufs=N_HD) as at_pool,
                tc.tile_pool(name="wo", bufs=8) as wo_pool,
                tc.tile_pool(name="osb", bufs=4) as o_pool,
                tc.tile_pool(name="opsum", bufs=1, space="PSUM") as opsum,
            ):
                # transpose A rows (our slice) into lhsT tiles [hd, s]
                at_tiles = []
                for m in range(N_HD):
                    j, mm = divmod(m, Q_PER_CORE)
                    at = at_pool.tile([128, S_SLICE], BF16, tag="at")
                    at_tiles.append(at)
                    for sc in range(2):
                        nc.sync.dma_start_transpose(
                            out=at[:, 128 * sc:128 * (sc + 1)],
                            in_=a2a_out[j, 128 * sc:128 * (sc + 1),
                                        128 * mm:128 * (mm + 1)])

                for sc in range(2):
                    pso = opsum.tile([128, D_MODEL], F32, tag="pso")
                    for m in range(N_HD):
                        wo_sb = wo_pool.tile([128, D_MODEL], BF16, tag="wo")
                        nc.sync.dma_start(out=wo_sb[:],
                                          in_=woT[128 * m:128 * (m + 1), :])
                        for e in range(0, D_MODEL, 512):
                            nc.tensor.matmul(pso[:, e:e + 512],
                                             at_tiles[m][:, 128 * sc:128 * (sc + 1)],
                                             wo_sb[:, e:e + 512],
                                             start=(m == 0), stop=(m == N_HD - 1))
                    for e in range(0, D_MODEL, 512):
                        ob = o_pool.tile([128, 512], F32, tag="osb")
                        nc.vector.tensor_copy(ob[:], pso[:, e:e + 512])
                        nc.sync.dma_start(
                            out=out_slice[128 * sc:128 * (sc + 1), e:e + 512],
                            in_=ob[:])

    nc.compile()
    return nc


def _host_prep(x, Wq, Wk, Wv, Wo, q_norm_w, k_norm_w):
    bf = ml_dtypes.bfloat16
    x2 = np.asarray(x, np.float32).reshape(S, D_MODEL)
    xT = np.ascontiguousarray(x2.T).astype(bf)
    woT = np.ascontiguousarray(np.asarray(Wo, np.float32).T).astype(bf)

    # rope tables with norm weights + rotate-half sign folded in
    pos = np.arange(S, dtype=np.float64)
    inv_freq = 1.0 / (ROPE_THETA ** (np.arange(0, HEAD_DIM, 2, dtype=np.float64)
                                     / HEAD_DIM))
    ang = pos[:, None] * inv_freq[None, :]          # (S, 64)
    cos = np.concatenate([np.cos(ang), np.cos(ang)], axis=1)  # (S, 128)
    sin = np.concatenate([np.sin(ang), np.sin(ang)], axis=1)
    sgn = np.where(np.arange(HEAD_DIM) < 64, -1.0, 1.0)[None, :]
    rot_idx = (np.arange(HEAD_DIM) + 64) % HEAD_DIM
    qw = np.asarray(q_norm_w, np.float64)
    kw = np.asarray(k_norm_w, np.float64)
    cos_q = cos * qw[None, :]
    sin_q = sin * sgn * qw[rot_idx][None, :]
    cos_k = cos * kw[None, :]
    sin_k = sin * sgn * kw[rot_idx][None, :]
    cosT = np.concatenate([np.tile(cos_q, (1, Q_PER_CORE)), cos_k], axis=1).astype(bf)
    sinT = np.concatenate([np.tile(sin_q, (1, Q_PER_CORE)), sin_k], axis=1).astype(bf)

    tri = np.triu(np.ones((128, 128), np.float32)).astype(bf)  # keep p <= f

    Wq = np.asarray(Wq, np.float32)
    Wk = np.asarray(Wk, np.float32)
    Wv = np.asarray(Wv, np.float32)
    in_maps = []
    for i in range(N_CORES):
        w_i = np.concatenate([
            Wq[Q_PER_CORE * HEAD_DIM * i: Q_PER_CORE * HEAD_DIM * (i + 1)],
            Wk[HEAD_DIM * i: HEAD_DIM * (i + 1)],
            Wv[HEAD_DIM * i: HEAD_DIM * (i + 1)],
        ], axis=0)                                   # (768, 2560)
        wqkvT = np.ascontiguousarray(w_i.T).astype(bf)
        in_maps.append({
            "xT": xT, "wqkvT": wqkvT, "woT": woT,
            "cosT": cosT, "sinT": sinT, "tri": tri,
        })
    return in_maps


def kernel(x, Wq, Wk, Wv, Wo, q_norm_w, k_norm_w, _trace=False, _trace_out=None):
    if "nc" not in _CACHE:
        _CACHE["nc"] = _build()
    nc = _CACHE["nc"]
    in_maps = _host_prep(x, Wq, Wk, Wv, Wo, q_norm_w, k_norm_w)
    kw = {}
    if _trace:
        kw = dict(trace=True)
        if _trace_out:
            kw["tmpdir"] = _trace_out
    res = run_bass_kernel_spmd(nc, in_maps, list(range(N_CORES)), **kw)
    _CACHE["last_exec_ns"] = res.exec_time_ns
    r = res.results
    out = np.concatenate([r[i]["out_slice"] for i in range(N_CORES)], axis=0)
    out = out.reshape(1, S, D_MODEL)
    kc = np.stack([r[i]["k_cache"] for i in range(N_CORES)], axis=0)[None]
    vc = np.stack([r[i]["v_cache"] for i in range(N_CORES)], axis=0)[None]
    return (out, kc, vc)


# revision 5
# speedup vs baseline: 1.2755x; 1.2755x over previous
"""Distributed Trainium2 kernel for the attention block (8 NeuronCores).

Sharding: core i owns KV head i and Q heads [4i, 4i+4) (tensor parallel over
heads). Attention outputs are transposed on the fly and exchanged with an
AllToAll so each core computes the o_proj for its 256-row slice of the
sequence; host concatenates slices.

All matmuls run in bf16 with fp32 PSUM accumulation; softmax statistics and
norms in fp32. RoPE + QK RMS-norm are folded into precomputed tables and
per-row scales.
"""
from contextlib import ExitStack

import numpy as np
import ml_dtypes

import concourse.bass as bass
import concourse.bacc as bacc
import concourse.tile as tile
from concourse import mybir
from concourse.bass_utils import run_bass_kernel_spmd

BF16 = mybir.dt.bfloat16
F32 = mybir.dt.float32

N_CORES = 8
S = 2048
D_MODEL = 2560
HEAD_DIM = 128
NUM_HEADS = 32
NUM_KV_HEADS = 8
Q_PER_CORE = NUM_HEADS // N_CORES          # 4
QKV_R = Q_PER_CORE * HEAD_DIM + 2 * HEAD_DIM  # 768 rows per core (4q + k + v)
QK_R = Q_PER_CORE * HEAD_DIM + HEAD_DIM       # 640 (q + k, normed+roped)
ROPE_THETA = 5e6
EPS = 1e-6
N_ST = S // 128          # 16 seq tiles
N_DC = D_MODEL // 128    # 20 contraction chunks
S_SLICE = S // N_CORES   # 256 output rows per core
N_HD = (NUM_HEADS * HEAD_DIM) // 128  # 32 o_proj contraction chunks
AV_W = 132               # 128 v cols + 1 ones + 3 pad
E_HALF = D_MODEL // 2    # 1280

_CACHE = {}


def _build():
    nc = bacc.Bacc("TRN2", target_bir_lowering=False, debug=False,
                   num_devices=N_CORES)

    xT = nc.dram_tensor("xT", [D_MODEL, S], BF16, kind="ExternalInput")
    wqkvT = nc.dram_tensor("wqkvT", [D_MODEL, QKV_R], BF16, kind="ExternalInput")
    woT = nc.dram_tensor("woT", [NUM_HEADS * HEAD_DIM, D_MODEL], BF16,
                         kind="ExternalInput")
    cosT = nc.dram_tensor("cosT", [S, QK_R], BF16, kind="ExternalInput")
    sinT = nc.dram_tensor("sinT", [S, QK_R], BF16, kind="ExternalInput")
    tri = nc.dram_tensor("tri", [128, 128], BF16, kind="ExternalInput")

    out_slice = nc.dram_tensor("out_slice", [S_SLICE, D_MODEL], F32,
                               kind="ExternalOutput")
    k_cache = nc.dram_tensor("k_cache", [S, HEAD_DIM], F32, kind="ExternalOutput")
    v_cache = nc.dram_tensor("v_cache", [S, HEAD_DIM], F32, kind="ExternalOutput")

    with tile.TileContext(nc) as tc:
        with (
            tc.tile_pool(name="persist", bufs=1) as pp,
            tc.tile_pool(name="vaug", bufs=N_ST) as vaug_pool,
            tc.tile_pool(name="dram", bufs=1, space="DRAM") as dram,
        ):
            # persistent tiles
            qkT_all = pp.tile([128, 5, S], BF16)    # [d, block(4q+k), s]
            v_aug = [vaug_pool.tile([128, AV_W], BF16, tag="vaug", name=f"vaug{t}")
                     for t in range(N_ST)]
            tri_sb = pp.tile([128, 128], BF16)
            nc.sync.dma_start(out=tri_sb[:], in_=tri[:])
            eps_sb = pp.tile([128, 1], F32)
            nc.vector.memset(eps_sb, float(HEAD_DIM) * EPS)
            for t in range(N_ST):
                nc.gpsimd.memset(v_aug[t][:, 128:129], 1.0)
                nc.gpsimd.memset(v_aug[t][:, 129:AV_W], 0.0)

            a2a_in = dram.tile([N_CORES, Q_PER_CORE * HEAD_DIM, S_SLICE], BF16)
            a2a_out = dram.tile([N_CORES, Q_PER_CORE * HEAD_DIM, S_SLICE], BF16)

            # ---------------- Phase A: QKV projection + norm + rope ----------
            with (
                tc.tile_pool(name="xt", bufs=N_DC) as xt_pool,
                tc.tile_pool(name="wk", bufs=N_DC) as wk_pool,
                tc.tile_pool(name="tabs", bufs=3) as tab_pool,
                tc.tile_pool(name="qkwork", bufs=4) as work_pool,
                tc.tile_pool(name="stats", bufs=6) as stat_pool,
                tc.tile_pool(name="apsum", bufs=2, space="PSUM") as apsum,
            ):
                xT_sb = [xt_pool.tile([128, S], BF16, tag="xt", name=f"xTs{d}")
                         for d in range(N_DC)]
                wq_sb = [wk_pool.tile([128, QKV_R], BF16, tag="wk", name=f"wqs{d}")
                         for d in range(N_DC)]
                for d in range(N_DC):
                    nc.sync.dma_start(out=xT_sb[d][:], in_=xT[128 * d:128 * (d + 1), :])
                    nc.sync.dma_start(out=wq_sb[d][:], in_=wqkvT[128 * d:128 * (d + 1), :])

                for t in range(N_ST):
                    ps = apsum.tile([128, QKV_R], F32, tag="qkvps")
                    for d in range(N_DC):
                        lhsT = xT_sb[d][:, 128 * t:128 * (t + 1)]
                        nc.tensor.matmul(ps[:, 0:512], lhsT, wq_sb[d][:, 0:512],
                                         start=(d == 0), stop=(d == N_DC - 1))
                        nc.tensor.matmul(ps[:, 512:QKV_R], lhsT, wq_sb[d][:, 512:QKV_R],
                                         start=(d == 0), stop=(d == N_DC - 1))

                    # v: fp32 cache out + bf16 for AV
                    v32 = work_pool.tile([128, HEAD_DIM], F32, tag="v32")
                    nc.vector.tensor_copy(v32[:], ps[:, 640:768])
                    nc.gpsimd.dma_start(out=v_cache[128 * t:128 * (t + 1), :],
                                        in_=v32[:])
                    nc.vector.tensor_copy(v_aug[t][:, 0:128], v32[:])

                    # rms-norm stats for 4 q heads + k
                    sums = stat_pool.tile([128, 5], F32, tag="sums")
                    scr = work_pool.tile([128, 128], BF16, tag="sqscr")
                    for b in range(5):
                        nc.scalar.activation(
                            out=scr[:], in_=ps[:, 128 * b:128 * (b + 1)],
                            func=mybir.ActivationFunctionType.Square,
                            accum_out=sums[:, b:b + 1])
                    inv = stat_pool.tile([128, 5], F32, tag="inv")
                    nc.scalar.activation(out=inv[:], in_=sums[:],
                                         func=mybir.ActivationFunctionType.Sqrt,
                                         bias=eps_sb[:])
                    nc.vector.reciprocal(inv[:], inv[:])
                    # k scale needs * sqrt(HEAD_DIM) (q keeps 1/sqrt(d) fold)
                    nc.vector.tensor_scalar_mul(inv[:, 4:5], inv[:, 4:5],
                                                float(np.sqrt(HEAD_DIM)))

                    # evacuate q,k with norm scale applied -> bf16
                    qk_n = work_pool.tile([128, QK_R], BF16, tag="qkn")
                    for b in range(5):
                        nc.scalar.activation(
                            out=qk_n[:, 128 * b:128 * (b + 1)],
                            in_=ps[:, 128 * b:128 * (b + 1)],
                            func=mybir.ActivationFunctionType.Copy,
                            scale=inv[:, b:b + 1])

                    # rope
                    cos_sb = tab_pool.tile([128, QK_R], BF16, tag="cos")
                    sin_sb = tab_pool.tile([128, QK_R], BF16, tag="sin")
                    nc.sync.dma_start(out=cos_sb[:], in_=cosT[128 * t:128 * (t + 1), :])
                    nc.sync.dma_start(out=sin_sb[:], in_=sinT[128 * t:128 * (t + 1), :])
                    qk_r = work_pool.tile([128, QK_R], BF16, tag="qkr")
                    tmp = work_pool.tile([128, QK_R], BF16, tag="ropetmp")
                    qn3 = qk_n.rearrange("p (b h) -> p b h", b=5)
                    qr3 = qk_r.rearrange("p (b h) -> p b h", b=5)
                    sn3 = sin_sb.rearrange("p (b h) -> p b h", b=5)
                    nc.vector.tensor_mul(qr3[:, :, 0:64], qn3[:, :, 64:128],
                                         sn3[:, :, 0:64])
                    nc.vector.tensor_mul(qr3[:, :, 64:128], qn3[:, :, 0:64],
                                         sn3[:, :, 64:128])
                    nc.vector.tensor_mul(tmp[:], qk_n[:], cos_sb[:])
                    nc.vector.tensor_add(qk_r[:], qk_r[:], tmp[:])

                    # k cache (fp32)
                    k32 = work_pool.tile([128, HEAD_DIM], F32, tag="k32")
                    nc.vector.tensor_copy(k32[:], qk_r[:, 512:640])
                    nc.gpsimd.dma_start(out=k_cache[128 * t:128 * (t + 1), :],
                                        in_=k32[:])

                    # batched transpose: all 5 blocks in one xbar call
                    nc.sync.dma_start_transpose(
                        out=qkT_all[:, :, 128 * t:128 * (t + 1)], in_=qk_r[:])

            # ---------------- Phase B: attention + pre-A2A transpose --------
            wo_stack = ExitStack()
            wo_pool = wo_stack.enter_context(tc.tile_pool(name="wo", bufs=16))
            with (
                tc.tile_pool(name="pt", bufs=1) as pt_pool,
                tc.tile_pool(name="asb", bufs=6) as a_pool,
                tc.tile_pool(name="den", bufs=8) as den_pool,
                tc.tile_pool(name="spsum", bufs=4, space="PSUM") as spsum,
                tc.tile_pool(name="avpsum", bufs=4, space="PSUM") as avpsum,
            ):
                kT = qkT_all[:, 4, :]
                for h in range(Q_PER_CORE):
                    qT = qkT_all[:, h, :]
                    strips = []
                    for c in range(N_ST):
                        n_sq = S - 128 * c
                        pt = pt_pool.tile([128, n_sq], BF16, tag=f"pt{c}",
                                          name=f"pt{h}_{c}")
                        strips.append(pt)
                        for j in range(0, n_sq, 512):
                            w = min(512, n_sq - j)
                            sps = spsum.tile([128, 512], F32, tag="sps")
                            nc.tensor.matmul(sps[:, 0:w],
                                             kT[:, 128 * c:128 * (c + 1)],
                                             qT[:, 128 * c + j:128 * c + j + w],
                                             start=True, stop=True)
                            nc.scalar.activation(out=pt[:, j:j + w], in_=sps[:, 0:w],
                                                 func=mybir.ActivationFunctionType.Exp)
                        # causal mask on diagonal block (keep sk <= sq)
                        nc.gpsimd.tensor_mul(pt[:, 0:128], pt[:, 0:128], tri_sb[:])

                    for t in range(N_ST):
                        av = avpsum.tile([128, AV_W], F32, tag="av")
                        for c in range(t + 1):
                            nc.tensor.matmul(av[:],
                                             strips[c][:, 128 * (t - c):128 * (t - c + 1)],
                                             v_aug[c][:],
                                             start=(c == 0), stop=(c == t))
                        den = den_pool.tile([128, 1], F32, tag="den")
                        nc.vector.reciprocal(den[:], av[:, 128:129])
                        a_sb = a_pool.tile([128, HEAD_DIM], BF16, tag="asb")
                        nc.scalar.activation(out=a_sb[:], in_=av[:, 0:128],
                                             func=mybir.ActivationFunctionType.Copy,
                                             scale=den[:])
                        # transpose now (spread over phase B) so phase C needs none
                        at_sb = a_pool.tile([128, HEAD_DIM], BF16, tag="atsb")
                        eng = nc.sync if (h * N_ST + t) % 2 == 0 else nc.scalar
                        eng.dma_start_transpose(out=at_sb[:], in_=a_sb[:])
                        eng.dma_start(
                            out=a2a_in[t // 2,
                                       128 * h:128 * (h + 1),
                                       128 * (t % 2):128 * (t % 2 + 1)],
                            in_=at_sb[:])

                nc.gpsimd.collective_compute(
                    "AllToAll", mybir.AluOpType.bypass,
                    replica_groups=[list(range(N_CORES))],
                    ins=[a2a_in.opt()], outs=[a2a_out.opt()])

            # ---------------- Phase C: o_proj for our seq slice -------------
            with (
                tc.tile_pool(name="at", bufs=N_HD) as at_pool,
                tc.tile_pool(name="osb", bufs=4) as o_pool,
                tc.tile_pool(name="opsum", bufs=2, space="PSUM") as opsum,
            ):
                at_tiles = []
                for m in range(N_HD):
                    j, mm = divmod(m, Q_PER_CORE)
                    at = at_pool.tile([128, S_SLICE], BF16, tag="at",
                                      name=f"at{m}")
                    at_tiles.append(at)
                    nc.sync.dma_start(
                        out=at[:], in_=a2a_out[j, 128 * mm:128 * (mm + 1), :])

                for eh in range(2):
                    e0 = eh * E_HALF
                    psos = [opsum.tile([128, E_HALF], F32, tag="pso",
                                       name=f"pso{eh}_{sc}") for sc in range(2)]
                    for m in range(N_HD):
                        wo_sb = wo_pool.tile([128, E_HALF], BF16, tag="wo",
                                             name=f"wo{eh}_{m}")
                        nc.sync.dma_start(out=wo_sb[:],
                                          in_=woT[128 * m:128 * (m + 1),
                                                  e0:e0 + E_HALF])
                        for sc in range(2):
                            for e in range(0, E_HALF, 512):
                                w = min(512, E_HALF - e)
                                nc.tensor.matmul(
                                    psos[sc][:, e:e + w],
                                    at_tiles[m][:, 128 * sc:128 * (sc + 1)],
                                    wo_sb[:, e:e + w],
                                    start=(m == 0), stop=(m == N_HD - 1))
                    for sc in range(2):
                        for e in range(0, E_HALF, 640):
                            ob = o_pool.tile([128, 640], F32, tag="osb")
                            nc.vector.tensor_copy(ob[:], psos[sc][:, e:e + 640])
                            nc.sync.dma_start(
                                out=out_slice[128 * sc:128 * (sc + 1),
                                              e0 + e:e0 + e + 640],
                                in_=ob[:])
            wo_stack.close()

    nc.compile()
    return nc


def _host_prep(x, Wq, Wk, Wv, Wo, q_norm_w, k_norm_w):
    bf = ml_dtypes.bfloat16
    x2 = np.asarray(x, np.float32).reshape(S, D_MODEL)
    xT = np.ascontiguousarray(x2.T).astype(bf)
    woT = np.ascontiguousarray(np.asarray(Wo, np.float32).T).astype(bf)

    # rope tables with norm weights + rotate-half sign folded in
    pos = np.arange(S, dtype=np.float64)
    inv_freq = 1.0 / (ROPE_THETA ** (np.arange(0, HEAD_DIM, 2, dtype=np.float64)
                                     / HEAD_DIM))
    ang = pos[:, None] * inv_freq[None, :]          # (S, 64)
    cos = np.concatenate([np.cos(ang), np.cos(ang)], axis=1)  # (S, 128)
    sin = np.concatenate([np.sin(ang), np.sin(ang)], axis=1)
    sgn = np.where(np.arange(HEAD_DIM) < 64, -1.0, 1.0)[None, :]
    rot_idx = (np.arange(HEAD_DIM) + 64) % HEAD_DIM
    qw = np.asarray(q_norm_w, np.float64)
    kw = np.asarray(k_norm_w, np.float64)
    cos_q = cos * qw[None, :]
    sin_q = sin * sgn * qw[rot_idx][None, :]
    cos_k = cos * kw[None, :]
    sin_k = sin * sgn * kw[rot_idx][None, :]
    cosT = np.concatenate([np.tile(cos_q, (1, Q_PER_CORE)), cos_k], axis=1).astype(bf)
    sinT = np.concatenate([np.tile(sin_q, (1, Q_PER_CORE)), sin_k], axis=1).astype(bf)

    tri = np.triu(np.ones((128, 128), np.float32)).astype(bf)  # keep p <= f

    Wq = np.asarray(Wq, np.float32)
    Wk = np.asarray(Wk, np.float32)
    Wv = np.asarray(Wv, np.float32)
    in_maps = []
    for i in range(N_CORES):
        w_i = np.concatenate([
            Wq[Q_PER_CORE * HEAD_DIM * i: Q_PER_CORE * HEAD_DIM * (i + 1)],
            Wk[HEAD_DIM * i: HEAD_DIM * (i + 1)],
            Wv[HEAD_DIM * i: HEAD_DIM * (i + 1)],
        ], axis=0)                                   # (768, 2560)
        wqkvT = np.ascontiguousarray(w_i.T).astype(bf)
        in_maps.append({
            "xT": xT, "wqkvT": wqkvT, "woT": woT,
            "cosT": cosT, "sinT": sinT, "tri": tri,
        })
    return in_maps


def kernel(x, Wq, Wk, Wv, Wo, q_norm_w, k_norm_w, _trace=False, _trace_out=None):
    if "nc" not in _CACHE:
        _CACHE["nc"] = _build()
    nc = _CACHE["nc"]
    in_maps = _host_prep(x, Wq, Wk, Wv, Wo, q_norm_w, k_norm_w)
    kw = {}
    if _trace:
        kw = dict(trace=True)
        if _trace_out:
            kw["tmpdir"] = _trace_out
    res = run_bass_kernel_spmd(nc, in_maps, list(range(N_CORES)), **kw)
    _CACHE["last_exec_ns"] = res.exec_time_ns
    r = res.results
    out = np.concatenate([r[i]["out_slice"] for i in range(N_CORES)], axis=0)
    out = out.reshape(1, S, D_MODEL)
    kc = np.stack([r[i]["k_cache"] for i in range(N_CORES)], axis=0)[None]
    vc = np.stack([r[i]["v_cache"] for i in range(N_CORES)], axis=0)[None]
    return (out, kc, vc)
